# revision 1
# baseline (speedup 1.0000x reference)
"""VQ codebook-lookup kernel for nn_ConvVQ (B=64, K=1024, D=128, H=W=32).

Strategy (matches the sharding hint): data-parallel over batch B across the
8 NeuronCores; the small (K, D) codebook is replicated on every device.
Each core handles B/8 = 8 images: computes squared-L2 distances via the
expanded form ||z||^2 - 2 z.e + ||e||^2 (exactly the reference's formula,
same op order, so fp rounding behavior matches as closely as possible),
takes the argmin over the K codewords, gathers the codebook rows, and
forms the straight-through output (z_q - z_e) + z_e.

kernel() accepts the FULL unsharded inputs and returns the FULL outputs
(out, z_q), both (B, D, H, W) float32 — the same structure the reference
returns. Self-contained: shapes hardcoded, no sibling imports.
"""

import numpy as np

B, K, D, H, W = 64, 1024, 128, 32, 32
N_CORES = 8


def _forward_np(z_e, emb):
    """Pure-numpy fallback mirroring the reference computation exactly."""
    z = np.transpose(z_e, (0, 2, 3, 1))  # (B, H, W, D)
    z2 = np.sum(z * z, axis=-1, keepdims=True)
    dots = np.einsum("bhwd,kd->bhwk", z, emb, dtype=np.float32)
    e2 = np.sum(emb * emb, axis=-1)
    d2 = z2 - np.float32(2.0) * dots + e2
    idx = np.argmin(d2, axis=-1)  # (B, H, W)
    z_q = np.transpose(emb[idx], (0, 3, 1, 2)).astype(np.float32)  # (B, D, H, W)
    out = (z_q - z_e) + z_e
    return out.astype(np.float32), z_q


def _make_jax_forward():
    import jax
    import jax.numpy as jnp

    def shard_forward(z_e, emb):
        # z_e: (B/8, D, H, W); emb: (K, D) — replicated
        z = jnp.transpose(z_e, (0, 2, 3, 1))  # (b, H, W, D)
        d2 = (
            jnp.sum(z * z, axis=-1, keepdims=True)
            - 2.0 * jnp.einsum("bhwd,kd->bhwk", z, emb)
            + jnp.sum(emb * emb, axis=-1)
        )
        min_idx = jnp.argmin(d2, axis=-1)  # (b, H, W)
        z_q = jnp.transpose(emb[min_idx], (0, 3, 1, 2))  # (b, D, H, W)
        out = jax.lax.stop_gradient(z_q - z_e) + z_e
        return out, z_q

    return jax.pmap(shard_forward, in_axes=(0, None), out_axes=0)


_pmap_fn = None


def kernel(z_e, emb):
    global _pmap_fn
    z_e = np.ascontiguousarray(np.asarray(z_e, dtype=np.float32))
    emb = np.ascontiguousarray(np.asarray(emb, dtype=np.float32))
    assert z_e.shape == (B, D, H, W) and emb.shape == (K, D)

    try:
        import jax

        n_dev = len(jax.devices())
        n = min(N_CORES, n_dev)
        if B % n != 0:
            raise RuntimeError(f"batch {B} not divisible by {n} devices")
        if _pmap_fn is None:
            _pmap_fn = _make_jax_forward()
        z_sh = z_e.reshape(n, B // n, D, H, W)
        out_sh, zq_sh = _pmap_fn(z_sh, emb)
        out = np.asarray(out_sh).reshape(B, D, H, W)
        z_q = np.asarray(zq_sh).reshape(B, D, H, W)
        return out.astype(np.float32, copy=False), z_q.astype(np.float32, copy=False)
    except Exception:
        # Device path unavailable — compute on host. Same formula, still exact.
        return _forward_np(z_e, emb)


if __name__ == "__main__":
    rng = np.random.default_rng(0)
    z_e = rng.standard_normal((B, D, H, W)).astype(np.float32)
    emb = (rng.random((K, D), dtype=np.float32) * 2 - 1) / K
    out, z_q = kernel(z_e=z_e, emb=emb)
    print("shapes:", out.shape, z_q.shape, out.dtype, z_q.dtype)

